# revision 13
# baseline (speedup 1.0000x reference)
"""DenseFlashAttention (GNN segment-softmax attention) on 8 trn2 NeuronCores.

Sharding: receivers (and their incident edges) sharded across 8 cores; the
DxD weights are folded host-side (A = Wq Wk^T * D^-0.5, W2 = Wv Wo) and
replicated. Each core computes out rows for its 12500 receivers; host
gathers.

v2 design (memory-regime): per-edge data streams in fp8 (e3m4), twice —
once d-partitioned for the logits matmul, once edge-partitioned for the
value accumulation — so no Ve projection and no psum->sbuf re-orientation
copies are needed.

Device algorithm per core:
  - 12800 receiver slots in 800 groups of 16 (64 big groups with 384 edge
    slots, 736 small with 256); per-group edge lists padded with
    self-masking pad edges.
  - xs [68, ECAP] e3m4: rows 0..63 x[sender]^T, 64 ones, 65 s'=slot/4,
    66 q_hi=slot^2//16, 67 q_lo=(slot^2%16)/16 (pads: [1,0,14,0]).
  - per 128-edge tile: logits matmul lhsT=xs tile, rhs = per-group table
    [68,16] bf16 (rows 0..63 a_r = A^T x_r, 64 -C r^2, 65 8C r, 66/67
    -16C) -> S[e,r] = a_r.x_s - C(r-slot)^2 exactly (C=64).
  - one exp per psum bank (ACT): psum f32 -> p bf16 in sbuf.
  - accum matmul: lhsT = xsT tile [128,65] e3m4 (x cols + ones), rhs = p
    [128,16] bf16 -> [num|denom] [65,16] per group, accumulated in psum.
  - per 32-group bank: copy [65,512] -> sbuf bf16, then final projection
    for its 4 chunks of 128 slots: out = xloc + (num @ W2) / denom.
"""

import os
import time
from contextlib import ExitStack

import numpy as np
import ml_dtypes

# ---------------- static problem/config constants (hardcoded) ----------------
N = 100000
D = 64
E = 1600000
NCORES = 8
NLOC = N // NCORES            # 12500 receivers per core
RG = 16                       # receivers per group
NG = 800                      # groups per core
SLOTS = NG * RG               # 12800 receiver slots (300 pad receivers)
NBIG = 64                     # 3-tile groups (cap 384 edges)
NSMALL = NG - NBIG            # 2-tile groups (cap 256 edges)
GT_BIG = 3
GT_SMALL = 2
GB_EDGE = GT_BIG * 128        # 384
GS_EDGE = GT_SMALL * 128      # 256
ECAP = NBIG * GB_EDGE + NSMALL * GS_EDGE   # 212992 edge slots per core
NTILES = ECAP // 128          # 1664
ROWS = 68                     # 64 x rows + ones + s' + q_hi + q_lo
CMASK = 64.0                  # mask penalty coefficient
GPB = 32                      # groups per accum psum bank
NBANKS = NG // GPB            # 25
NCHUNK = SLOTS // 128         # 100 final projection chunks (4 per bank)
SCALE = D ** -0.5

_F32 = np.float32
_BF16 = ml_dtypes.bfloat16
_E3M4 = ml_dtypes.float8_e3m4


def _group_tiles(g):
    return GT_BIG if g < NBIG else GT_SMALL


def _group_edge_base(g):
    return g * GB_EDGE if g < NBIG else \
        NBIG * GB_EDGE + (g - NBIG) * GS_EDGE


def _group_tile_base(g):
    return g * GT_BIG if g < NBIG else \
        NBIG * GT_BIG + (g - NBIG) * GT_SMALL


def _bank_groups(b):
    return list(range(b * GPB, (b + 1) * GPB))


# ---------------- device kernel (built/compiled once) ----------------
_CACHE = {}


def _build_nc():
    import concourse.tile as tile
    from concourse import bacc, mybir

    f32 = mybir.dt.float32
    bf16 = mybir.dt.bfloat16
    e3m4 = mybir.dt.float8e3
    nc = bacc.Bacc("TRN2", target_bir_lowering=False, debug=False,
                   num_devices=NCORES)
    xs_ap = nc.dram_tensor("xs", [ROWS, ECAP], e3m4,
                           kind="ExternalInput").ap()
    xst_ap = nc.dram_tensor("xst", [128, NTILES * 65], e3m4,
                            kind="ExternalInput").ap()
    tbl_ap = nc.dram_tensor("tbl", [ROWS, NG * RG], bf16,
                            kind="ExternalInput").ap()
    w2_ap = nc.dram_tensor("w2", [D, D], bf16, kind="ExternalInput").ap()
    xloc_ap = nc.dram_tensor("xloc", [128, NCHUNK * D], bf16,
                             kind="ExternalInput").ap()
    out_ap = nc.dram_tensor("out", [128, NCHUNK * D], bf16,
                            kind="ExternalOutput").ap()

    EXP = mybir.ActivationFunctionType.Exp
    COPY = mybir.ActivationFunctionType.Copy

    # DMA chunks: pairs of banks (last chunk single). Bank b tiles/edges:
    def bank_tile_base(b):
        return _group_tile_base(b * GPB)

    def bank_edge_base(b):
        return _group_edge_base(b * GPB)

    chunks = [(b, 1) for b in range(NBANKS)]

    with tile.TileContext(nc) as tc:
        with ExitStack() as octx:
            const_pool = octx.enter_context(tc.tile_pool(name="const",
                                                         bufs=1))
            w2_sb = const_pool.tile([D, D], bf16)
            nc.sync.dma_start(w2_sb[:], w2_ap[:, :])
            one_sb = const_pool.tile([65, 1], bf16)
            nc.vector.memset(one_sb[:], 1.0)

            stream_pool = octx.enter_context(
                tc.tile_pool(name="stream", bufs=5))
            p_pool = octx.enter_context(tc.tile_pool(name="pp", bufs=4))
            odn_pool = octx.enter_context(tc.tile_pool(name="odn", bufs=3))
            fin_sb = octx.enter_context(tc.tile_pool(name="finsb", bufs=3))
            psL = octx.enter_context(
                tc.tile_pool(name="psL", bufs=3, space="PSUM"))
            psN = octx.enter_context(
                tc.tile_pool(name="psN", bufs=2, space="PSUM"))
            fin_ps = octx.enter_context(
                tc.tile_pool(name="finps", bufs=2, space="PSUM"))

            def emit_final(b, odn_b):
                # 4 chunks of 128 receiver slots for bank b:
                # out = xloc + (num @ W2) / denom
                xt = fin_sb.tile([128, 4 * D], bf16, tag="xt")
                nc.sync.dma_start(xt[:], xloc_ap[:, b * 4 * D:(b + 1) * 4 * D])
                ps_c = fin_ps.tile([128, 288], f32, tag="psc")
                for j in range(4):
                    nc.tensor.matmul(
                        out=ps_c[:, j * 8:j * 8 + 1],
                        lhsT=odn_b[64:65, j * 128:(j + 1) * 128],
                        rhs=one_sb[64:65, :], start=True, stop=True)
                dn = fin_sb.tile([128, 4], f32, tag="dn")
                nc.vector.tensor_scalar_add(
                    dn[:].rearrange("p (c o) -> p c o", o=1),
                    ps_c[:, 0:32].rearrange("p (c o) -> p c o", o=8)[:, :, 0:1],
                    1e-30)
                rec = fin_sb.tile([128, 4], f32, tag="rec")
                nc.vector.reciprocal(rec[:], dn[:])
                out_sb = fin_sb.tile([128, 4 * D], bf16, tag="osb")
                for j in range(4):
                    nc.tensor.matmul(
                        out=ps_c[:, 32 + j * D:32 + (j + 1) * D],
                        lhsT=odn_b[0:64, j * 128:(j + 1) * 128],
                        rhs=w2_sb[:], start=True, stop=True)
                    sc = fin_sb.tile([128, D], f32, tag="sc")
                    if j % 2 == 0:
                        nc.scalar.activation(
                            sc[:], ps_c[:, 32 + j * D:32 + (j + 1) * D],
                            COPY, scale=rec[:, j:j + 1])
                    else:
                        nc.vector.tensor_scalar_mul(
                            sc[:], ps_c[:, 32 + j * D:32 + (j + 1) * D],
                            rec[:, j:j + 1])
                    nc.gpsimd.tensor_add(
                        out_sb[:, j * D:(j + 1) * D],
                        xt[:, j * D:(j + 1) * D], sc[:])
                nc.scalar.dma_start(
                    out_ap[:, b * 4 * D:(b + 1) * 4 * D], out_sb[:])

            # pending: deferred accumulation for software pipelining
            # entry: (bank, sub_tiles, p_sb, ps_nd, is_last_sub, odn_b)
            pending = []

            def drain_one():
                b, sub_tiles, p_sb, ps_nd, last, odn_b, xst_c, tb0 = \
                    pending.pop(0)
                # per group in this sub: accumulate [num|denom]
                lt = 0
                while lt < len(sub_tiles):
                    g, t0 = sub_tiles[lt]
                    gt = _group_tiles(g)
                    gl = g - b * GPB
                    for t in range(gt):
                        gtile = _group_tile_base(g) + t
                        lcol = (gtile - tb0) * 65
                        nc.tensor.matmul(
                            out=ps_nd[:, gl * RG:(gl + 1) * RG],
                            lhsT=xst_c[:, lcol:lcol + 65],
                            rhs=p_sb[:, (lt + t) * RG:(lt + t + 1) * RG],
                            start=(t == 0), stop=(t == gt - 1))
                    lt += gt
                if last:
                    nc.vector.tensor_copy(odn_b[:], ps_nd[:])
                    emit_final(b, odn_b)

            for b0, nb in chunks:
                e0 = bank_edge_base(b0)
                e1 = bank_edge_base(b0 + nb) if b0 + nb < NBANKS else ECAP
                t0 = bank_tile_base(b0)
                t1 = bank_tile_base(b0 + nb) if b0 + nb < NBANKS else NTILES
                xs_c = stream_pool.tile([ROWS, GPB * GB_EDGE], e3m4,
                                        tag="xs")
                nc.sync.dma_start(xs_c[:, :e1 - e0], xs_ap[:, e0:e1])
                xst_c = stream_pool.tile([128, GPB * GT_BIG * 65], e3m4,
                                         tag="xst")
                nc.sync.dma_start(xst_c[:, :(t1 - t0) * 65],
                                  xst_ap[:, t0 * 65:t1 * 65])
                tbl_c = stream_pool.tile([ROWS, GPB * RG], bf16,
                                         tag="tbl")
                nc.sync.dma_start(tbl_c[:, :nb * GPB * RG],
                                  tbl_ap[:, b0 * GPB * RG:
                                         (b0 + nb) * GPB * RG])
                for b in range(b0, b0 + nb):
                    odn_b = odn_pool.tile([65, GPB * RG], bf16, tag="odn")
                    ps_nd = psN.tile([65, GPB * RG], f32, tag="psnd")
                    groups = _bank_groups(b)
                    nsub = 4 if b * GPB < NBIG else 2
                    gps = GPB // nsub        # groups per sub
                    for s in range(nsub):
                        sgroups = groups[s * gps:(s + 1) * gps]
                        sub_tiles = []       # (group, tile_in_group)
                        for g in sgroups:
                            for t in range(_group_tiles(g)):
                                sub_tiles.append((g, t))
                        ncols = len(sub_tiles) * RG
                        psl = psL.tile([128, 512], f32, tag="psl")
                        for lt, (g, t) in enumerate(sub_tiles):
                            gtile = _group_tile_base(g) + t
                            ecol = (gtile - t0) * 128
                            nc.tensor.matmul(
                                out=psl[:, lt * RG:(lt + 1) * RG],
                                lhsT=xs_c[:, ecol:ecol + 128],
                                rhs=tbl_c[:, (g - b0 * GPB) * RG:
                                          (g - b0 * GPB + 1) * RG],
                                start=True, stop=True)
                        p_sb = p_pool.tile([128, 512], bf16, tag="p")
                        nc.scalar.activation(p_sb[:, :ncols],
                                             psl[:, :ncols], EXP)
                        pending.append((b, sub_tiles, p_sb, ps_nd,
                                        s == nsub - 1, odn_b, xst_c, t0))
                        if len(pending) > 1:
                            drain_one()
            while pending:
                drain_one()

    nc.compile()
    return nc


def _get_nc():
    if "nc" not in _CACHE:
        t0 = time.time()
        _CACHE["nc"] = _build_nc()
        print(f"[kernel] bass trace+compile: {time.time()-t0:.1f}s",
              flush=True)
    return _CACHE["nc"]


# ---------------- host-side sharding / preprocessing ----------------

def _pack_groups(deg):
    """Assign SLOTS receivers (incl. pads) to groups: NBIG big (<=384
    edges) + NSMALL small (<=256), RG receivers each. Returns
    (grp_of, slot_of)."""
    order = np.argsort(-deg, kind="stable")
    bin_of = np.empty(SLOTS, np.int64)
    slot_of = np.empty(SLOTS, np.int64)
    idx = np.arange(SLOTS)
    bin_of[order] = idx % NG
    slot_of[order] = idx // NG
    bsum = np.bincount(bin_of, weights=deg.astype(np.float64), minlength=NG)
    # biggest-sum bins become the big class (groups 0..NBIG-1)
    rank = np.argsort(-bsum, kind="stable")
    perm = np.empty(NG, np.int64)
    perm[rank] = np.arange(NG)
    grp_of = perm[bin_of]
    gsum = np.bincount(grp_of, weights=deg.astype(np.float64), minlength=NG)

    members = [list(np.where(grp_of == g)[0]) for g in range(NG)]
    it = 0
    while True:
        small_over = [g for g in range(NBIG, NG) if gsum[g] > GS_EDGE]
        big_over = [g for g in range(NBIG) if gsum[g] > GB_EDGE]
        if not small_over and not big_over:
            break
        if small_over:
            gs = small_over[0]
            gb = int(np.argmin(gsum[:NBIG]))
            hs = max(members[gs], key=lambda r: deg[r])
            lb = min(members[gb], key=lambda r: deg[r])
        else:
            gb = big_over[0]
            gs = NBIG + int(np.argmin(gsum[NBIG:]))
            hs = min(members[gs], key=lambda r: deg[r])
            lb = max(members[gb], key=lambda r: deg[r])
        members[gs].remove(hs)
        members[gb].remove(lb)
        members[gs].append(lb)
        members[gb].append(hs)
        grp_of[hs], grp_of[lb] = gb, gs
        slot_of[hs], slot_of[lb] = slot_of[lb], slot_of[hs]
        gsum[gs] += deg[lb] - deg[hs]
        gsum[gb] += deg[hs] - deg[lb]
        it += 1
        assert it < 50000, "bin-pack repair failed"
    return grp_of, slot_of


def _prep_core(x, sender, receiver, A, W2, core):
    """Build xs/xst/tbl/xloc arrays + slot map for one core."""
    lo = core * NLOC
    mask = (receiver >= lo) & (receiver < lo + NLOC)
    snd = sender[mask]
    rcv = receiver[mask] - lo

    deg = np.bincount(rcv, minlength=SLOTS)  # pads 12500..12799 have deg 0
    grp_of, slot_of = _pack_groups(deg)

    egrp = grp_of[rcv]
    eorder = np.argsort(egrp, kind="stable")
    cnt = np.bincount(egrp, minlength=NG)
    cum = np.concatenate([[0], np.cumsum(cnt)[:-1]])
    ofs = np.arange(len(eorder)) - np.repeat(cum, cnt)
    gbase = np.array([_group_edge_base(g) for g in range(NG)], np.int64)
    col = gbase[egrp[eorder]] + ofs

    slot_e = slot_of[rcv[eorder]].astype(np.int64)   # receiver slot per edge

    xs = np.zeros((ROWS, ECAP), _F32)
    xs[:D, col] = x[snd[eorder]].T
    xs[D, :] = 1.0
    # pad defaults: s'=0, q_hi=14, q_lo=0  (penalty <= -64*224)
    sp = np.zeros(ECAP, _F32)
    qhi = np.full(ECAP, 14.0, _F32)
    qlo = np.zeros(ECAP, _F32)
    sp[col] = slot_e / 4.0
    q = slot_e * slot_e
    qhi[col] = (q // 16).astype(_F32)
    qlo[col] = (q % 16).astype(_F32) / 16.0
    xs[D + 1] = sp
    xs[D + 2] = qhi
    xs[D + 3] = qlo
    xs8 = xs.astype(_E3M4)

    # edge-partitioned value stream: per tile [128, 65]
    xt = np.zeros((ECAP, 65), _F32)
    xt[col, :D] = x[snd[eorder]]
    xt[col, D] = 1.0
    xst = np.ascontiguousarray(
        xt.reshape(NTILES, 128, 65).transpose(1, 0, 2).reshape(
            128, NTILES * 65)).astype(_E3M4)

    slot_id = grp_of * RG + slot_of
    xr = np.zeros((SLOTS, D), _F32)
    xr[slot_id[:NLOC]] = x[lo:lo + NLOC]
    xr_cm = np.ascontiguousarray(
        xr.reshape(NCHUNK, 128, D).transpose(1, 0, 2).reshape(
            128, NCHUNK * D)).astype(_BF16)

    tbl = np.zeros((ROWS, NG * RG), _F32)
    t3 = tbl.reshape(ROWS, NG, RG)
    av = (A.T @ xr.T).astype(_F32)          # [D, SLOTS]
    t3[0:D] = av.reshape(D, NG, RG)
    r = np.arange(RG, dtype=_F32)
    t3[D, :, :] = -CMASK * r * r
    t3[D + 1, :, :] = 8.0 * CMASK * r
    t3[D + 2, :, :] = -16.0 * CMASK
    t3[D + 3, :, :] = -16.0 * CMASK
    tbl16 = tbl.astype(_BF16)

    return xs8, xst, tbl16, xr_cm, slot_id


def kernel(x, edge_index, Wq, Wk, Wv, Wo, **_unused):
    x = np.asarray(x, _F32)
    edge_index = np.asarray(edge_index)
    Wq = np.asarray(Wq, _F32)
    Wk = np.asarray(Wk, _F32)
    Wv = np.asarray(Wv, _F32)
    Wo = np.asarray(Wo, _F32)
    sender = np.asarray(edge_index[0], np.int64)
    receiver = np.asarray(edge_index[1], np.int64)

    A = (Wq @ Wk.T).astype(_F32) * _F32(SCALE)
    W2 = (Wv @ Wo).astype(_F32)
    w2_16 = W2.astype(_BF16)

    nc = _get_nc()

    in_maps = []
    slot_ids = []
    t0 = time.time()
    for c in range(NCORES):
        xs8, xst, tbl16, xr_cm, slot_id = _prep_core(
            x, sender, receiver, A, W2, c)
        in_maps.append({"xs": xs8, "xst": xst, "tbl": tbl16,
                        "w2": w2_16, "xloc": xr_cm})
        slot_ids.append(slot_id)
    print(f"[kernel] host prep: {time.time()-t0:.1f}s", flush=True)

    from concourse import bass_utils
    trace = bool(int(os.environ.get("KERNEL_TRACE", "0")))
    t0 = time.time()
    res = bass_utils.run_bass_kernel_spmd(
        nc, in_maps, core_ids=list(range(NCORES)), trace=trace)
    print(f"[kernel] device run: {time.time()-t0:.1f}s", flush=True)
    _CACHE["last_results"] = res

    out = np.empty((N, D), _F32)
    for c in range(NCORES):
        dev = res.results[c]["out"].astype(_F32).reshape(128, NCHUNK, D)
        dev = dev.transpose(1, 0, 2).reshape(SLOTS, D)
        out[c * NLOC:(c + 1) * NLOC] = dev[slot_ids[c][:NLOC]]
    return out


# revision 22
# speedup vs baseline: 1.2516x; 1.2516x over previous
"""DenseFlashAttention (GNN segment-softmax attention) on 8 trn2 NeuronCores.

Sharding: receivers (and their incident edges) sharded across 8 cores; the
DxD weights are folded host-side (A = Wq Wk^T * D^-0.5, W2 = Wv Wo) and
replicated. Each core computes out rows for its 12500 receivers; host
gathers.

v2 design (memory-regime): per-edge data streams in fp8 (e3m4), twice —
once d-partitioned for the logits matmul, once edge-partitioned for the
value accumulation — so no Ve projection and no psum->sbuf re-orientation
copies are needed.

Device algorithm per core:
  - 12800 receiver slots in 800 groups of 16 (64 big groups with 384 edge
    slots, 736 small with 256); per-group edge lists padded with
    self-masking pad edges.
  - xs [68, ECAP] e3m4: rows 0..63 x[sender]^T, 64 ones, 65 s'=slot/4,
    66 q_hi=slot^2//16, 67 q_lo=(slot^2%16)/16 (pads: [1,0,14,0]).
  - per 128-edge tile: logits matmul lhsT=xs tile, rhs = per-group table
    [68,16] bf16 (rows 0..63 a_r = A^T x_r, 64 -C r^2, 65 8C r, 66/67
    -16C) -> S[e,r] = a_r.x_s - C(r-slot)^2 exactly (C=64).
  - one exp per psum bank (ACT): psum f32 -> p bf16 in sbuf.
  - accum matmul: lhsT = xsT tile [128,65] e3m4 (x cols + ones), rhs = p
    [128,16] bf16 -> [num|denom] [65,16] per group, accumulated in psum.
  - per 32-group bank: copy [65,512] -> sbuf bf16, then final projection
    for its 4 chunks of 128 slots: out = xloc + (num @ W2) / denom.
"""

import os
import time
from contextlib import ExitStack

import numpy as np
import ml_dtypes

# ---------------- static problem/config constants (hardcoded) ----------------
N = 100000
D = 64
E = 1600000
NCORES = 8
NLOC = N // NCORES            # 12500 receivers per core
RG = 16                       # receivers per group
NG = 800                      # groups per core
SLOTS = NG * RG               # 12800 receiver slots (300 pad receivers)
NBIG = 64                     # 3-tile groups (cap 384 edges)
NSMALL = NG - NBIG            # 2-tile groups (cap 256 edges)
GT_BIG = 3
GT_SMALL = 2
GB_EDGE = GT_BIG * 128        # 384
GS_EDGE = GT_SMALL * 128      # 256
ECAP = NBIG * GB_EDGE + NSMALL * GS_EDGE   # 212992 edge slots per core
NTILES = ECAP // 128          # 1664
ROWS = 68                     # 64 x rows + ones + s' + q_hi + q_lo
CMASK = 64.0                  # mask penalty coefficient
GPB = 32                      # groups per accum psum bank
NBANKS = NG // GPB            # 25
NCHUNK = SLOTS // 128         # 100 final projection chunks (4 per bank)
SCALE = D ** -0.5

_F32 = np.float32
_BF16 = ml_dtypes.bfloat16
_E3M4 = ml_dtypes.float8_e3m4


def _group_tiles(g):
    return GT_BIG if g < NBIG else GT_SMALL


def _group_edge_base(g):
    return g * GB_EDGE if g < NBIG else \
        NBIG * GB_EDGE + (g - NBIG) * GS_EDGE


def _group_tile_base(g):
    return g * GT_BIG if g < NBIG else \
        NBIG * GT_BIG + (g - NBIG) * GT_SMALL


def _bank_groups(b):
    return list(range(b * GPB, (b + 1) * GPB))


# ---------------- device kernel (built/compiled once) ----------------
_CACHE = {}


def _build_nc():
    import concourse.tile as tile
    from concourse import bacc, mybir

    f32 = mybir.dt.float32
    bf16 = mybir.dt.bfloat16
    e3m4 = mybir.dt.float8e3
    nc = bacc.Bacc("TRN2", target_bir_lowering=False, debug=False,
                   num_devices=NCORES)
    xs_ap = nc.dram_tensor("xs", [ROWS, ECAP], e3m4,
                           kind="ExternalInput").ap()
    xst_ap = nc.dram_tensor("xst", [128, NTILES * 65], e3m4,
                            kind="ExternalInput").ap()
    tbl_ap = nc.dram_tensor("tbl", [ROWS, NG * RG], bf16,
                            kind="ExternalInput").ap()
    w2_ap = nc.dram_tensor("w2", [D, D], bf16, kind="ExternalInput").ap()
    xloc_ap = nc.dram_tensor("xloc", [128, NCHUNK * D], bf16,
                             kind="ExternalInput").ap()
    out_ap = nc.dram_tensor("out", [128, NCHUNK * D], bf16,
                            kind="ExternalOutput").ap()

    EXP = mybir.ActivationFunctionType.Exp
    COPY = mybir.ActivationFunctionType.Copy

    # DMA chunks: pairs of banks (last chunk single). Bank b tiles/edges:
    def bank_tile_base(b):
        return _group_tile_base(b * GPB)

    def bank_edge_base(b):
        return _group_edge_base(b * GPB)

    chunks = [(cc * 2, min(2, NBANKS - cc * 2))
              for cc in range((NBANKS + 1) // 2)]

    with tile.TileContext(nc) as tc:
        with ExitStack() as octx:
            const_pool = octx.enter_context(tc.tile_pool(name="const",
                                                         bufs=1))
            w2_sb = const_pool.tile([D, D], bf16)
            nc.sync.dma_start(w2_sb[:], w2_ap[:, :])
            one_sb = const_pool.tile([65, 1], bf16)
            nc.vector.memset(one_sb[:], 1.0)

            stream_pool = octx.enter_context(
                tc.tile_pool(name="stream", bufs=4))
            p_pool = octx.enter_context(tc.tile_pool(name="pp", bufs=4))
            odn_pool = octx.enter_context(tc.tile_pool(name="odn", bufs=3))
            fin_sb = octx.enter_context(tc.tile_pool(name="finsb", bufs=3))
            psL = octx.enter_context(
                tc.tile_pool(name="psL", bufs=3, space="PSUM"))
            psN = octx.enter_context(
                tc.tile_pool(name="psN", bufs=2, space="PSUM"))
            fin_ps = octx.enter_context(
                tc.tile_pool(name="finps", bufs=2, space="PSUM"))

            def emit_final(b, odn_b, xt):
                # 4 chunks of 128 receiver slots for bank b:
                # out = xloc + (num @ W2) / denom
                ps_c = fin_ps.tile([128, 288], f32, tag="psc")
                for j in range(4):
                    nc.tensor.matmul(
                        out=ps_c[:, j * 8:j * 8 + 1],
                        lhsT=odn_b[64:65, j * 128:(j + 1) * 128],
                        rhs=one_sb[64:65, :], start=True, stop=True)
                dn = fin_sb.tile([128, 4], f32, tag="dn")
                nc.vector.tensor_scalar_add(
                    dn[:].rearrange("p (c o) -> p c o", o=1),
                    ps_c[:, 0:32].rearrange("p (c o) -> p c o", o=8)[:, :, 0:1],
                    1e-30)
                rec = fin_sb.tile([128, 4], f32, tag="rec")
                nc.vector.reciprocal(rec[:], dn[:])
                out_sb = fin_sb.tile([128, 4 * D], bf16, tag="osb")
                for j in range(4):
                    nc.tensor.matmul(
                        out=ps_c[:, 32 + j * D:32 + (j + 1) * D],
                        lhsT=odn_b[0:64, j * 128:(j + 1) * 128],
                        rhs=w2_sb[:], start=True, stop=True)
                    sc = fin_sb.tile([128, D], f32, tag="sc")
                    if j % 2 == 0:
                        nc.scalar.activation(
                            sc[:], ps_c[:, 32 + j * D:32 + (j + 1) * D],
                            COPY, scale=rec[:, j:j + 1])
                    else:
                        nc.vector.tensor_scalar_mul(
                            sc[:], ps_c[:, 32 + j * D:32 + (j + 1) * D],
                            rec[:, j:j + 1])
                    nc.gpsimd.tensor_add(
                        out_sb[:, j * D:(j + 1) * D],
                        xt[:, j * D:(j + 1) * D], sc[:])
                nc.gpsimd.dma_start(
                    out_ap[:, b * 4 * D:(b + 1) * 4 * D], out_sb[:])

            # pending: deferred accumulation for software pipelining
            # entry: (bank, sub_tiles, p_sb, ps_nd, is_last_sub, odn_b)
            pending = []

            def drain_one():
                b, sub_tiles, p_sb, ps_nd, last, odn_b, xst_c, tb0, xt = \
                    pending.pop(0)
                # per group in this sub: accumulate [num|denom]
                lt = 0
                while lt < len(sub_tiles):
                    g, t0 = sub_tiles[lt]
                    gt = _group_tiles(g)
                    gl = g - b * GPB
                    for t in range(gt):
                        gtile = _group_tile_base(g) + t
                        lcol = (gtile - tb0) * 65
                        nc.tensor.matmul(
                            out=ps_nd[:, gl * RG:(gl + 1) * RG],
                            lhsT=xst_c[:, lcol:lcol + 65],
                            rhs=p_sb[:, (lt + t) * RG:(lt + t + 1) * RG],
                            start=(t == 0), stop=(t == gt - 1))
                    lt += gt
                if last:
                    nc.vector.tensor_copy(odn_b[:], ps_nd[:])
                    emit_final(b, odn_b, xt)

            for b0, nb in chunks:
                e0 = bank_edge_base(b0)
                e1 = bank_edge_base(b0 + nb) if b0 + nb < NBANKS else ECAP
                t0 = bank_tile_base(b0)
                t1 = bank_tile_base(b0 + nb) if b0 + nb < NBANKS else NTILES
                xs_c = stream_pool.tile([ROWS, 2 * GPB * GB_EDGE], e3m4,
                                        tag="xs")
                nc.sync.dma_start(xs_c[:, :e1 - e0], xs_ap[:, e0:e1])
                xst_c = stream_pool.tile([128, 2 * GPB * GT_BIG * 65], e3m4,
                                         tag="xst")
                nc.sync.dma_start(xst_c[:, :(t1 - t0) * 65],
                                  xst_ap[:, t0 * 65:t1 * 65])
                tbl_c = stream_pool.tile([ROWS, 2 * GPB * RG], bf16,
                                         tag="tbl")
                nc.sync.dma_start(tbl_c[:, :nb * GPB * RG],
                                  tbl_ap[:, b0 * GPB * RG:
                                         (b0 + nb) * GPB * RG])
                for b in range(b0, b0 + nb):
                    odn_b = odn_pool.tile([65, GPB * RG], bf16, tag="odn")
                    ps_nd = psN.tile([65, GPB * RG], f32, tag="psnd")
                    xt = fin_sb.tile([128, 4 * D], bf16, tag="xt")
                    nc.gpsimd.dma_start(
                        xt[:], xloc_ap[:, b * 4 * D:(b + 1) * 4 * D])
                    groups = _bank_groups(b)
                    nsub = 4 if b * GPB < NBIG else 2
                    gps = GPB // nsub        # groups per sub
                    for s in range(nsub):
                        sgroups = groups[s * gps:(s + 1) * gps]
                        sub_tiles = []       # (group, tile_in_group)
                        for g in sgroups:
                            for t in range(_group_tiles(g)):
                                sub_tiles.append((g, t))
                        ncols = len(sub_tiles) * RG
                        psl = psL.tile([128, 512], f32, tag="psl")
                        for lt, (g, t) in enumerate(sub_tiles):
                            gtile = _group_tile_base(g) + t
                            ecol = (gtile - t0) * 128
                            nc.tensor.matmul(
                                out=psl[:, lt * RG:(lt + 1) * RG],
                                lhsT=xs_c[:, ecol:ecol + 128],
                                rhs=tbl_c[:, (g - b0 * GPB) * RG:
                                          (g - b0 * GPB + 1) * RG],
                                start=True, stop=True)
                        p_sb = p_pool.tile([128, 512], bf16, tag="p")
                        nc.scalar.activation(p_sb[:, :ncols],
                                             psl[:, :ncols], EXP)
                        pending.append((b, sub_tiles, p_sb, ps_nd,
                                        s == nsub - 1, odn_b, xst_c, t0, xt))
                        if len(pending) > 1:
                            drain_one()
            while pending:
                drain_one()

    nc.compile()
    return nc


def _get_nc():
    if "nc" not in _CACHE:
        t0 = time.time()
        _CACHE["nc"] = _build_nc()
        print(f"[kernel] bass trace+compile: {time.time()-t0:.1f}s",
              flush=True)
    return _CACHE["nc"]


# ---------------- host-side sharding / preprocessing ----------------

def _pack_groups(deg):
    """Assign SLOTS receivers (incl. pads) to groups: NBIG big (<=384
    edges) + NSMALL small (<=256), RG receivers each. Returns
    (grp_of, slot_of)."""
    order = np.argsort(-deg, kind="stable")
    bin_of = np.empty(SLOTS, np.int64)
    slot_of = np.empty(SLOTS, np.int64)
    idx = np.arange(SLOTS)
    bin_of[order] = idx % NG
    slot_of[order] = idx // NG
    bsum = np.bincount(bin_of, weights=deg.astype(np.float64), minlength=NG)
    # biggest-sum bins become the big class (groups 0..NBIG-1)
    rank = np.argsort(-bsum, kind="stable")
    perm = np.empty(NG, np.int64)
    perm[rank] = np.arange(NG)
    grp_of = perm[bin_of]
    gsum = np.bincount(grp_of, weights=deg.astype(np.float64), minlength=NG)

    members = [list(np.where(grp_of == g)[0]) for g in range(NG)]
    it = 0
    while True:
        small_over = [g for g in range(NBIG, NG) if gsum[g] > GS_EDGE]
        big_over = [g for g in range(NBIG) if gsum[g] > GB_EDGE]
        if not small_over and not big_over:
            break
        if small_over:
            gs = small_over[0]
            gb = int(np.argmin(gsum[:NBIG]))
            hs = max(members[gs], key=lambda r: deg[r])
            lb = min(members[gb], key=lambda r: deg[r])
        else:
            gb = big_over[0]
            gs = NBIG + int(np.argmin(gsum[NBIG:]))
            hs = min(members[gs], key=lambda r: deg[r])
            lb = max(members[gb], key=lambda r: deg[r])
        members[gs].remove(hs)
        members[gb].remove(lb)
        members[gs].append(lb)
        members[gb].append(hs)
        grp_of[hs], grp_of[lb] = gb, gs
        slot_of[hs], slot_of[lb] = slot_of[lb], slot_of[hs]
        gsum[gs] += deg[lb] - deg[hs]
        gsum[gb] += deg[hs] - deg[lb]
        it += 1
        assert it < 50000, "bin-pack repair failed"
    return grp_of, slot_of


def _prep_core(x, sender, receiver, A, W2, core):
    """Build xs/xst/tbl/xloc arrays + slot map for one core."""
    lo = core * NLOC
    mask = (receiver >= lo) & (receiver < lo + NLOC)
    snd = sender[mask]
    rcv = receiver[mask] - lo

    deg = np.bincount(rcv, minlength=SLOTS)  # pads 12500..12799 have deg 0
    grp_of, slot_of = _pack_groups(deg)

    egrp = grp_of[rcv]
    eorder = np.argsort(egrp, kind="stable")
    cnt = np.bincount(egrp, minlength=NG)
    cum = np.concatenate([[0], np.cumsum(cnt)[:-1]])
    ofs = np.arange(len(eorder)) - np.repeat(cum, cnt)
    gbase = np.array([_group_edge_base(g) for g in range(NG)], np.int64)
    col = gbase[egrp[eorder]] + ofs

    slot_e = slot_of[rcv[eorder]].astype(np.int64)   # receiver slot per edge

    xs = np.zeros((ROWS, ECAP), _F32)
    xs[:D, col] = x[snd[eorder]].T
    xs[D, :] = 1.0
    # pad defaults: s'=0, q_hi=14, q_lo=0  (penalty <= -64*224)
    sp = np.zeros(ECAP, _F32)
    qhi = np.full(ECAP, 14.0, _F32)
    qlo = np.zeros(ECAP, _F32)
    sp[col] = slot_e / 4.0
    q = slot_e * slot_e
    qhi[col] = (q // 16).astype(_F32)
    qlo[col] = (q % 16).astype(_F32) / 16.0
    xs[D + 1] = sp
    xs[D + 2] = qhi
    xs[D + 3] = qlo
    xs8 = xs.astype(_E3M4)

    # edge-partitioned value stream: per tile [128, 65]
    xt = np.zeros((ECAP, 65), _F32)
    xt[col, :D] = x[snd[eorder]]
    xt[col, D] = 1.0
    xst = np.ascontiguousarray(
        xt.reshape(NTILES, 128, 65).transpose(1, 0, 2).reshape(
            128, NTILES * 65)).astype(_E3M4)

    slot_id = grp_of * RG + slot_of
    xr = np.zeros((SLOTS, D), _F32)
    xr[slot_id[:NLOC]] = x[lo:lo + NLOC]
    xr_cm = np.ascontiguousarray(
        xr.reshape(NCHUNK, 128, D).transpose(1, 0, 2).reshape(
            128, NCHUNK * D)).astype(_BF16)

    tbl = np.zeros((ROWS, NG * RG), _F32)
    t3 = tbl.reshape(ROWS, NG, RG)
    av = (A.T @ xr.T).astype(_F32)          # [D, SLOTS]
    t3[0:D] = av.reshape(D, NG, RG)
    r = np.arange(RG, dtype=_F32)
    t3[D, :, :] = -CMASK * r * r
    t3[D + 1, :, :] = 8.0 * CMASK * r
    t3[D + 2, :, :] = -16.0 * CMASK
    t3[D + 3, :, :] = -16.0 * CMASK
    tbl16 = tbl.astype(_BF16)

    return xs8, xst, tbl16, xr_cm, slot_id


def kernel(x, edge_index, Wq, Wk, Wv, Wo, **_unused):
    x = np.asarray(x, _F32)
    edge_index = np.asarray(edge_index)
    Wq = np.asarray(Wq, _F32)
    Wk = np.asarray(Wk, _F32)
    Wv = np.asarray(Wv, _F32)
    Wo = np.asarray(Wo, _F32)
    sender = np.asarray(edge_index[0], np.int64)
    receiver = np.asarray(edge_index[1], np.int64)

    A = (Wq @ Wk.T).astype(_F32) * _F32(SCALE)
    W2 = (Wv @ Wo).astype(_F32)
    w2_16 = W2.astype(_BF16)

    nc = _get_nc()

    in_maps = []
    slot_ids = []
    t0 = time.time()
    for c in range(NCORES):
        xs8, xst, tbl16, xr_cm, slot_id = _prep_core(
            x, sender, receiver, A, W2, c)
        in_maps.append({"xs": xs8, "xst": xst, "tbl": tbl16,
                        "w2": w2_16, "xloc": xr_cm})
        slot_ids.append(slot_id)
    print(f"[kernel] host prep: {time.time()-t0:.1f}s", flush=True)

    from concourse import bass_utils
    trace = bool(int(os.environ.get("KERNEL_TRACE", "0")))
    t0 = time.time()
    res = bass_utils.run_bass_kernel_spmd(
        nc, in_maps, core_ids=list(range(NCORES)), trace=trace)
    print(f"[kernel] device run: {time.time()-t0:.1f}s", flush=True)
    _CACHE["last_results"] = res

    out = np.empty((N, D), _F32)
    for c in range(NCORES):
        dev = res.results[c]["out"].astype(_F32).reshape(128, NCHUNK, D)
        dev = dev.transpose(1, 0, 2).reshape(SLOTS, D)
        out[c * NLOC:(c + 1) * NLOC] = dev[slot_ids[c][:NLOC]]
    return out


# revision 28
# speedup vs baseline: 1.3018x; 1.0401x over previous
"""DenseFlashAttention (GNN segment-softmax attention) on 8 trn2 NeuronCores.

Sharding: receivers (and their incident edges) sharded across 8 cores; the
DxD weights are folded host-side (A = Wq Wk^T * D^-0.5, W2 = Wv Wo) and
replicated. Each core computes out rows for its 12500 receivers; host
gathers.

v2 design (memory-regime): per-edge data streams in fp8 (e3m4), twice —
once d-partitioned for the logits matmul, once edge-partitioned for the
value accumulation — so no Ve projection and no psum->sbuf re-orientation
copies are needed.

Device algorithm per core:
  - 12800 receiver slots in 800 groups of 16 (64 big groups with 384 edge
    slots, 736 small with 256); per-group edge lists padded with
    self-masking pad edges.
  - xs [68, ECAP] e3m4: rows 0..63 x[sender]^T, 64 ones, 65 s'=slot/4,
    66 q_hi=slot^2//16, 67 q_lo=(slot^2%16)/16 (pads: [1,0,14,0]).
  - per 128-edge tile: logits matmul lhsT=xs tile, rhs = per-group table
    [68,16] bf16 (rows 0..63 a_r = A^T x_r, 64 -C r^2, 65 8C r, 66/67
    -16C) -> S[e,r] = a_r.x_s - C(r-slot)^2 exactly (C=64).
  - one exp per psum bank (ACT): psum f32 -> p bf16 in sbuf.
  - accum matmul: lhsT = xsT tile [128,65] e3m4 (x cols + ones), rhs = p
    [128,16] bf16 -> [num|denom] [65,16] per group, accumulated in psum.
  - per 32-group bank: copy [65,512] -> sbuf bf16, then final projection
    for its 4 chunks of 128 slots: out = xloc + (num @ W2) / denom.
"""

import os
import time
from contextlib import ExitStack

import numpy as np
import ml_dtypes

# ---------------- static problem/config constants (hardcoded) ----------------
N = 100000
D = 64
E = 1600000
NCORES = 8
NLOC = N // NCORES            # 12500 receivers per core
RG = 16                       # receivers per group
NG = 800                      # groups per core
SLOTS = NG * RG               # 12800 receiver slots (300 pad receivers)
NBIG = 64                     # 3-tile groups (cap 384 edges)
NSMALL = NG - NBIG            # 2-tile groups (cap 256 edges)
GT_BIG = 3
GT_SMALL = 2
GB_EDGE = GT_BIG * 128        # 384
GS_EDGE = GT_SMALL * 128      # 256
ECAP = NBIG * GB_EDGE + NSMALL * GS_EDGE   # 212992 edge slots per core
NTILES = ECAP // 128          # 1664
ROWS = 68                     # 64 x rows + ones + s' + q_hi + q_lo
CMASK = 64.0                  # mask penalty coefficient
GPB = 32                      # groups per accum psum bank
NBANKS = NG // GPB            # 25
NCHUNK = SLOTS // 128         # 100 final projection chunks (4 per bank)
SCALE = D ** -0.5

_F32 = np.float32
_BF16 = ml_dtypes.bfloat16
_E3M4 = ml_dtypes.float8_e3m4


def _group_tiles(g):
    return GT_BIG if g < NBIG else GT_SMALL


def _group_edge_base(g):
    return g * GB_EDGE if g < NBIG else \
        NBIG * GB_EDGE + (g - NBIG) * GS_EDGE


def _group_tile_base(g):
    return g * GT_BIG if g < NBIG else \
        NBIG * GT_BIG + (g - NBIG) * GT_SMALL


def _bank_groups(b):
    return list(range(b * GPB, (b + 1) * GPB))


# ---------------- device kernel (built/compiled once) ----------------
_CACHE = {}


def _build_nc():
    import concourse.tile as tile
    from concourse import bacc, mybir

    f32 = mybir.dt.float32
    bf16 = mybir.dt.bfloat16
    e3m4 = mybir.dt.float8e3
    nc = bacc.Bacc("TRN2", target_bir_lowering=False, debug=False,
                   num_devices=NCORES)
    xs_ap = nc.dram_tensor("xs", [ROWS, ECAP], e3m4,
                           kind="ExternalInput").ap()
    xst_ap = nc.dram_tensor("xst", [128, NTILES * 65], e3m4,
                            kind="ExternalInput").ap()
    tbl_ap = nc.dram_tensor("tbl", [ROWS, NG * RG], bf16,
                            kind="ExternalInput").ap()
    w2_ap = nc.dram_tensor("w2", [D, D], bf16, kind="ExternalInput").ap()
    xloc_ap = nc.dram_tensor("xloc", [128, NCHUNK * D], bf16,
                             kind="ExternalInput").ap()
    out_ap = nc.dram_tensor("out", [128, NCHUNK * D], bf16,
                            kind="ExternalOutput").ap()

    EXP = mybir.ActivationFunctionType.Exp
    COPY = mybir.ActivationFunctionType.Copy

    # DMA chunks: pairs of banks (last chunk single). Bank b tiles/edges:
    def bank_tile_base(b):
        return _group_tile_base(b * GPB)

    def bank_edge_base(b):
        return _group_edge_base(b * GPB)

    chunks = [(cc * 2, min(2, NBANKS - cc * 2))
              for cc in range((NBANKS + 1) // 2)]

    with tile.TileContext(nc) as tc:
        with ExitStack() as octx:
            const_pool = octx.enter_context(tc.tile_pool(name="const",
                                                         bufs=1))
            w2_sb = const_pool.tile([D, D], bf16)
            nc.sync.dma_start(w2_sb[:], w2_ap[:, :])
            one_sb = const_pool.tile([65, 1], bf16)
            nc.vector.memset(one_sb[:], 1.0)

            stream_pool = octx.enter_context(
                tc.tile_pool(name="stream", bufs=4))
            p_pool = octx.enter_context(tc.tile_pool(name="pp", bufs=4))
            odn_pool = octx.enter_context(tc.tile_pool(name="odn", bufs=3))
            fin_sb = octx.enter_context(tc.tile_pool(name="finsb", bufs=3))
            psL = octx.enter_context(
                tc.tile_pool(name="psL", bufs=3, space="PSUM"))
            psN = octx.enter_context(
                tc.tile_pool(name="psN", bufs=2, space="PSUM"))
            fin_ps = octx.enter_context(
                tc.tile_pool(name="finps", bufs=2, space="PSUM"))

            def emit_final(b, odn_b, xt):
                # 4 chunks of 128 receiver slots for bank b:
                # out = xloc + (num @ W2) / denom
                ps_c = fin_ps.tile([128, 288], f32, tag="psc")
                for j in range(4):
                    nc.tensor.matmul(
                        out=ps_c[:, 32 + j * D:32 + (j + 1) * D],
                        lhsT=odn_b[0:64, j * 128:(j + 1) * 128],
                        rhs=w2_sb[:], start=True, stop=True)
                for j in range(4):
                    nc.tensor.matmul(
                        out=ps_c[:, j * 8:j * 8 + 1],
                        lhsT=odn_b[64:65, j * 128:(j + 1) * 128],
                        rhs=one_sb[64:65, :], start=True, stop=True)
                dn = fin_sb.tile([128, 4], f32, tag="dn")
                nc.vector.tensor_scalar_add(
                    dn[:].rearrange("p (c o) -> p c o", o=1),
                    ps_c[:, 0:32].rearrange("p (c o) -> p c o", o=8)[:, :, 0:1],
                    1e-30)
                rec = fin_sb.tile([128, 4], f32, tag="rec")
                nc.vector.reciprocal(rec[:], dn[:])
                out_sb = fin_sb.tile([128, 4 * D], bf16, tag="osb")
                for j in range(4):
                    sc = fin_sb.tile([128, D], f32, tag=f"sc{j % 2}")
                    if j % 2 == 0:
                        nc.scalar.activation(
                            sc[:], ps_c[:, 32 + j * D:32 + (j + 1) * D],
                            COPY, scale=rec[:, j:j + 1])
                    else:
                        nc.vector.tensor_scalar_mul(
                            sc[:], ps_c[:, 32 + j * D:32 + (j + 1) * D],
                            rec[:, j:j + 1])
                    nc.gpsimd.tensor_add(
                        out_sb[:, j * D:(j + 1) * D],
                        xt[:, j * D:(j + 1) * D], sc[:])
                nc.gpsimd.dma_start(
                    out_ap[:, b * 4 * D:(b + 1) * 4 * D], out_sb[:])

            # pending: deferred accumulation for software pipelining
            # entry: (bank, sub_tiles, p_sb, ps_nd, is_last_sub, odn_b)
            pending = []
            finq = []

            def drain_one():
                b, sub_tiles, p_sb, ps_nd, last, odn_b, xst_c, tb0, xt = \
                    pending.pop(0)
                # per group in this sub: accumulate [num|denom]
                lt = 0
                while lt < len(sub_tiles):
                    g, t0 = sub_tiles[lt]
                    gt = _group_tiles(g)
                    gl = g - b * GPB
                    for t in range(gt):
                        gtile = _group_tile_base(g) + t
                        lcol = (gtile - tb0) * 65
                        nc.tensor.matmul(
                            out=ps_nd[:, gl * RG:(gl + 1) * RG],
                            lhsT=xst_c[:, lcol:lcol + 65],
                            rhs=p_sb[:, (lt + t) * RG:(lt + t + 1) * RG],
                            start=(t == 0), stop=(t == gt - 1))
                    lt += gt
                if last:
                    nc.vector.tensor_copy(odn_b[:], ps_nd[:])
                    finq.append((b, odn_b, xt))

            for b0, nb in chunks:
                e0 = bank_edge_base(b0)
                e1 = bank_edge_base(b0 + nb) if b0 + nb < NBANKS else ECAP
                t0 = bank_tile_base(b0)
                t1 = bank_tile_base(b0 + nb) if b0 + nb < NBANKS else NTILES
                xs_c = stream_pool.tile([ROWS, 2 * GPB * GB_EDGE], e3m4,
                                        tag="xs")
                nc.sync.dma_start(xs_c[:, :e1 - e0], xs_ap[:, e0:e1])
                xst_c = stream_pool.tile([128, 2 * GPB * GT_BIG * 65], e3m4,
                                         tag="xst")
                nc.sync.dma_start(xst_c[:, :(t1 - t0) * 65],
                                  xst_ap[:, t0 * 65:t1 * 65])
                tbl_c = stream_pool.tile([ROWS, 2 * GPB * RG], bf16,
                                         tag="tbl")
                nc.sync.dma_start(tbl_c[:, :nb * GPB * RG],
                                  tbl_ap[:, b0 * GPB * RG:
                                         (b0 + nb) * GPB * RG])
                for b in range(b0, b0 + nb):
                    odn_b = odn_pool.tile([65, GPB * RG], bf16, tag="odn")
                    ps_nd = psN.tile([65, GPB * RG], f32, tag="psnd")
                    xt = fin_sb.tile([128, 4 * D], bf16, tag="xt")
                    nc.gpsimd.dma_start(
                        xt[:], xloc_ap[:, b * 4 * D:(b + 1) * 4 * D])
                    groups = _bank_groups(b)
                    nsub = 4 if b * GPB < NBIG else 2
                    gps = GPB // nsub        # groups per sub
                    for s in range(nsub):
                        sgroups = groups[s * gps:(s + 1) * gps]
                        sub_tiles = []       # (group, tile_in_group)
                        for g in sgroups:
                            for t in range(_group_tiles(g)):
                                sub_tiles.append((g, t))
                        ncols = len(sub_tiles) * RG
                        psl = psL.tile([128, 512], f32, tag="psl")
                        for lt, (g, t) in enumerate(sub_tiles):
                            gtile = _group_tile_base(g) + t
                            ecol = (gtile - t0) * 128
                            nc.tensor.matmul(
                                out=psl[:, lt * RG:(lt + 1) * RG],
                                lhsT=xs_c[:, ecol:ecol + 128],
                                rhs=tbl_c[:, (g - b0 * GPB) * RG:
                                          (g - b0 * GPB + 1) * RG],
                                start=True, stop=True)
                        p_sb = p_pool.tile([128, 512], bf16, tag="p")
                        nc.scalar.activation(p_sb[:, :ncols],
                                             psl[:, :ncols], EXP)
                        pending.append((b, sub_tiles, p_sb, ps_nd,
                                        s == nsub - 1, odn_b, xst_c, t0, xt))
                        if len(pending) > 1:
                            drain_one()
                        if finq:
                            emit_final(*finq.pop(0))
            while pending:
                drain_one()
            while finq:
                emit_final(*finq.pop(0))

    nc.compile()
    return nc


def _get_nc():
    if "nc" not in _CACHE:
        t0 = time.time()
        _CACHE["nc"] = _build_nc()
        print(f"[kernel] bass trace+compile: {time.time()-t0:.1f}s",
              flush=True)
    return _CACHE["nc"]


# ---------------- host-side sharding / preprocessing ----------------

def _pack_groups(deg):
    """Assign SLOTS receivers (incl. pads) to groups: NBIG big (<=384
    edges) + NSMALL small (<=256), RG receivers each. Returns
    (grp_of, slot_of)."""
    order = np.argsort(-deg, kind="stable")
    bin_of = np.empty(SLOTS, np.int64)
    slot_of = np.empty(SLOTS, np.int64)
    idx = np.arange(SLOTS)
    bin_of[order] = idx % NG
    slot_of[order] = idx // NG
    bsum = np.bincount(bin_of, weights=deg.astype(np.float64), minlength=NG)
    # biggest-sum bins become the big class (groups 0..NBIG-1)
    rank = np.argsort(-bsum, kind="stable")
    perm = np.empty(NG, np.int64)
    perm[rank] = np.arange(NG)
    grp_of = perm[bin_of]
    gsum = np.bincount(grp_of, weights=deg.astype(np.float64), minlength=NG)

    members = [list(np.where(grp_of == g)[0]) for g in range(NG)]
    it = 0
    while True:
        small_over = [g for g in range(NBIG, NG) if gsum[g] > GS_EDGE]
        big_over = [g for g in range(NBIG) if gsum[g] > GB_EDGE]
        if not small_over and not big_over:
            break
        if small_over:
            gs = small_over[0]
            gb = int(np.argmin(gsum[:NBIG]))
            hs = max(members[gs], key=lambda r: deg[r])
            lb = min(members[gb], key=lambda r: deg[r])
        else:
            gb = big_over[0]
            gs = NBIG + int(np.argmin(gsum[NBIG:]))
            hs = min(members[gs], key=lambda r: deg[r])
            lb = max(members[gb], key=lambda r: deg[r])
        members[gs].remove(hs)
        members[gb].remove(lb)
        members[gs].append(lb)
        members[gb].append(hs)
        grp_of[hs], grp_of[lb] = gb, gs
        slot_of[hs], slot_of[lb] = slot_of[lb], slot_of[hs]
        gsum[gs] += deg[lb] - deg[hs]
        gsum[gb] += deg[hs] - deg[lb]
        it += 1
        assert it < 50000, "bin-pack repair failed"
    return grp_of, slot_of


def _prep_core(x, sender, receiver, A, W2, core):
    """Build xs/xst/tbl/xloc arrays + slot map for one core."""
    lo = core * NLOC
    mask = (receiver >= lo) & (receiver < lo + NLOC)
    snd = sender[mask]
    rcv = receiver[mask] - lo

    deg = np.bincount(rcv, minlength=SLOTS)  # pads 12500..12799 have deg 0
    grp_of, slot_of = _pack_groups(deg)

    egrp = grp_of[rcv]
    eorder = np.argsort(egrp, kind="stable")
    cnt = np.bincount(egrp, minlength=NG)
    cum = np.concatenate([[0], np.cumsum(cnt)[:-1]])
    ofs = np.arange(len(eorder)) - np.repeat(cum, cnt)
    gbase = np.array([_group_edge_base(g) for g in range(NG)], np.int64)
    col = gbase[egrp[eorder]] + ofs

    slot_e = slot_of[rcv[eorder]].astype(np.int64)   # receiver slot per edge

    xs = np.zeros((ROWS, ECAP), _F32)
    xs[:D, col] = x[snd[eorder]].T
    xs[D, :] = 1.0
    # pad defaults: s'=0, q_hi=14, q_lo=0  (penalty <= -64*224)
    sp = np.zeros(ECAP, _F32)
    qhi = np.full(ECAP, 14.0, _F32)
    qlo = np.zeros(ECAP, _F32)
    sp[col] = slot_e / 4.0
    q = slot_e * slot_e
    qhi[col] = (q // 16).astype(_F32)
    qlo[col] = (q % 16).astype(_F32) / 16.0
    xs[D + 1] = sp
    xs[D + 2] = qhi
    xs[D + 3] = qlo
    xs8 = xs.astype(_E3M4)

    # edge-partitioned value stream: per tile [128, 65]
    xt = np.zeros((ECAP, 65), _F32)
    xt[col, :D] = x[snd[eorder]]
    xt[col, D] = 1.0
    xst = np.ascontiguousarray(
        xt.reshape(NTILES, 128, 65).transpose(1, 0, 2).reshape(
            128, NTILES * 65)).astype(_E3M4)

    slot_id = grp_of * RG + slot_of
    xr = np.zeros((SLOTS, D), _F32)
    xr[slot_id[:NLOC]] = x[lo:lo + NLOC]
    xr_cm = np.ascontiguousarray(
        xr.reshape(NCHUNK, 128, D).transpose(1, 0, 2).reshape(
            128, NCHUNK * D)).astype(_BF16)

    tbl = np.zeros((ROWS, NG * RG), _F32)
    t3 = tbl.reshape(ROWS, NG, RG)
    av = (A.T @ xr.T).astype(_F32)          # [D, SLOTS]
    t3[0:D] = av.reshape(D, NG, RG)
    r = np.arange(RG, dtype=_F32)
    t3[D, :, :] = -CMASK * r * r
    t3[D + 1, :, :] = 8.0 * CMASK * r
    t3[D + 2, :, :] = -16.0 * CMASK
    t3[D + 3, :, :] = -16.0 * CMASK
    tbl16 = tbl.astype(_BF16)

    return xs8, xst, tbl16, xr_cm, slot_id


def kernel(x, edge_index, Wq, Wk, Wv, Wo, **_unused):
    x = np.asarray(x, _F32)
    edge_index = np.asarray(edge_index)
    Wq = np.asarray(Wq, _F32)
    Wk = np.asarray(Wk, _F32)
    Wv = np.asarray(Wv, _F32)
    Wo = np.asarray(Wo, _F32)
    sender = np.asarray(edge_index[0], np.int64)
    receiver = np.asarray(edge_index[1], np.int64)

    A = (Wq @ Wk.T).astype(_F32) * _F32(SCALE)
    W2 = (Wv @ Wo).astype(_F32)
    w2_16 = W2.astype(_BF16)

    nc = _get_nc()

    in_maps = []
    slot_ids = []
    t0 = time.time()
    for c in range(NCORES):
        xs8, xst, tbl16, xr_cm, slot_id = _prep_core(
            x, sender, receiver, A, W2, c)
        in_maps.append({"xs": xs8, "xst": xst, "tbl": tbl16,
                        "w2": w2_16, "xloc": xr_cm})
        slot_ids.append(slot_id)
    print(f"[kernel] host prep: {time.time()-t0:.1f}s", flush=True)

    from concourse import bass_utils
    trace = bool(int(os.environ.get("KERNEL_TRACE", "0")))
    t0 = time.time()
    res = bass_utils.run_bass_kernel_spmd(
        nc, in_maps, core_ids=list(range(NCORES)), trace=trace)
    print(f"[kernel] device run: {time.time()-t0:.1f}s", flush=True)
    _CACHE["last_results"] = res

    out = np.empty((N, D), _F32)
    for c in range(NCORES):
        dev = res.results[c]["out"].astype(_F32).reshape(128, NCHUNK, D)
        dev = dev.transpose(1, 0, 2).reshape(SLOTS, D)
        out[c * NLOC:(c + 1) * NLOC] = dev[slot_ids[c][:NLOC]]
    return out
